# revision 1
# baseline (speedup 1.0000x reference)
"""MeanShiftClusterer Trainium2 Bass kernel (8 NeuronCores, SPMD).

Algorithm (reference: 10 mean-shift iterations + mode snap):
  iter:  K[i,j] = exp(-(|y_i - q_j|^2)/h^2) * w_j ; y <- (K@q)/rowsum(K)
  final: density[i] = rowsum(K(y,y)*w) ; snap each i to argmax_j density[j]
         over j within one bandwidth.

Device mapping per core (1024 query rows each, all 8192 sources):
  - Pairwise exponent via ONE K~98 fp16 matmul: hi/lo split rows at
    partition offsets {0,32,64,96} (engine partition-alignment), constant
    and c_j terms folded into spare rows -> plain Exp on ACT, ~1.4e-4 abs err.
  - ACT exp (fp16 out, scaled by 2^10 for fp16 headroom) -> B matmul
    ([q,1] fp16 stationary) accumulates numerator+denominator in PSUM.
  - Final: AllGather 8KB fp16 payload, density pass (same pipeline,
    ones stationary), AllGather density, mask+argmax via STT + max_index.
Host: trivial layout prep (centering, hi/lo splits) + final y[idx] gather.
"""
import sys
import numpy as np

sys.path.insert(0, '/opt/trn_rl_repo')

import concourse.bacc as bacc
import concourse.tile as tile
import concourse.mybir as mybir
import concourse.bass_isa as bass_isa
from concourse import bass_utils
from contextlib import ExitStack

dt = mybir.dt
F32, F16, U32 = dt.float32, dt.float16, dt.uint32
AF = mybir.ActivationFunctionType
OP = mybir.AluOpType

N = 8192
NC = 8
M = N // NC              # 1024 local query rows per core
H2 = 0.05 * 0.05
LN_C = float(np.log(2.0 ** 10))
N_ITERS = 10
NCH = N // 128           # 64 j-chunks
GRP = 3                  # j-chunks per ACT group (3 banks * 2 bufs + 2 acc = 8)
KP = 98                  # padded contraction dim (rows at 0..2,3,32..34,35,64..66,96,97)

_cache = {}


def _split16(x):
    h = x.astype(np.float16)
    l = (x.astype(np.float32) - h.astype(np.float32)).astype(np.float16)
    return h, l


def build_kernel(n_iters=N_ITERS, skip_final=False, final_reps=1):
    nc = bacc.Bacc("TRN2", target_bir_lowering=False, num_devices=NC)

    d_qs = nc.dram_tensor("qs", [KP, N], F16, kind="ExternalInput")
    d_q1 = nc.dram_tensor("q1", [128, 36 * NCH], F16, kind="ExternalInput")
    d_ya0 = nc.dram_tensor("ya0", [KP, M], F16, kind="ExternalInput")
    d_lnw = nc.dram_tensor("lnwloc", [1, M], F32, kind="ExternalInput")  # ln w + LN_C, local slice

    o_y = nc.dram_tensor("y_out", [3, M], F32, kind="ExternalOutput")
    o_idx = nc.dram_tensor("idx_out", [128, M // 128], U32, kind="ExternalOutput")

    with tile.TileContext(nc) as tc, ExitStack() as ctx:
        per = ctx.enter_context(tc.tile_pool(name="per", bufs=1))         # persistents
        dram = ctx.enter_context(tc.tile_pool(name="dram", bufs=1, space="DRAM"))

        # ---- persistent SBUF ----
        t_qs = per.tile([KP, N], F16)
        t_q1 = per.tile([128, 36 * NCH], F16)
        t_ya = per.tile([KP, M], F16)
        t_lnw = per.tile([1, M], F32)
        t_ones4 = per.tile([128, 36], F16)
        r_last = per.tile([1, M], F32)
        rl_last = per.tile([1, M], F16)
        t_qs2 = per.tile([KP, N], F16)    # density-pass stationary (j side, scaled)
        t_rhs2 = per.tile([KP, N], F16)   # mask-pass moving (j side, unscaled)
        t_qs2l = per.tile([KP, M], F16)   # mask-pass stationary (i side, scaled)
        t_densb = per.tile([128, N], F32)
        t_cand = per.tile([128, N], F32)
        t_rpt = per.tile([128, M // 128], F32)
        t_thr = per.tile([128, M // 128], F32)
        t_yf = per.tile([3, M], F32)
        t_idxall = per.tile([128, M // 128], U32)
        t_max8 = per.tile([128, 8], F32)
        t_idx8 = per.tile([128, 8], U32)

        # DRAM bounce buffers for collectives
        ag1_in = dram.tile([3, N], F16)
        ag1_out = dram.tile([3 * NC, N], F16)
        ag3_in = dram.tile([1, M], F32)
        ag3_out = dram.tile([NC, M], F32)
        rpt_dram = dram.tile([1, M], F32)

        nc.gpsimd.dma_start(t_qs[:], d_qs[:])
        nc.gpsimd.dma_start(t_q1[:], d_q1[:])
        nc.gpsimd.dma_start(t_lnw[:], d_lnw[:])
        nc.vector.memset(t_ones4[:], 1.0)
        nc.vector.memset(t_qs2[:], 0.0)
        nc.vector.memset(t_rhs2[:], 0.0)
        nc.vector.memset(t_qs2l[:], 0.0)

        nc.gpsimd.dma_start(t_ya[:], d_ya0[:])

        def pairwise_pass(lhs_a, rhs_ya, lhs_b, acc_into, kg_pool, ps_pool, half):
            """64-chunk j loop: A matmul -> exp -> B matmul accumulate.

            lhs_a: [KP, N] stationary (exponent terms), rhs_ya: [KP, 512] moving,
            lhs_b: [128, >=4] B stationary source, acc_into: psum [4, 512].
            """
            ngrp = (NCH + GRP - 1) // GRP

            def emit_b(g, kg):
                nch = min(GRP, NCH - g * GRP)
                for k in range(nch):
                    t = g * GRP + k
                    nc.tensor.matmul(acc_into,
                                     lhs_b[:, t * 36:(t + 1) * 36] if lhs_b is not None
                                     else t_ones4[:],
                                     kg[:, k * 512:(k + 1) * 512],
                                     start=(t == 0), stop=(t == NCH - 1))

            # depth-2 software pipeline: emit A(g)+ACT(g), then B(g-2) —
            # B(g-2)'s ACT finished two groups ago, so the in-order PE queue
            # never stalls on ACT.
            pending = []
            for g in range(ngrp):
                nch = min(GRP, NCH - g * GRP)
                wid = nch * 512
                pg = ps_pool.tile([128, GRP * 512], F32, tag="pg")
                kg = kg_pool.tile([128, GRP * 512], F16, tag="kg")
                for k in range(nch):
                    t = g * GRP + k
                    nc.tensor.matmul(pg[:, k * 512:(k + 1) * 512],
                                     t_qs[:, t * 128:(t + 1) * 128] if lhs_a is None
                                     else lhs_a[:, t * 128:(t + 1) * 128],
                                     rhs_ya, start=True, stop=True)
                nc.scalar.activation(kg[:, 0:wid], pg[:, 0:wid], AF.Exp,
                                     bias=0.0, scale=1.0)
                pending.append((g, kg))
                if len(pending) > 2:
                    emit_b(*pending.pop(0))
            for p in pending:
                emit_b(*p)

        # ==================== mean-shift iterations ====================
        assert n_iters % 2 == 0
        with tc.For_i(0, n_iters // 2, 1, hint_engines=(mybir.EngineType.PE,)):
            with tc.tile_pool(name="it_sb", bufs=2) as sbp, \
                 tc.tile_pool(name="it_kg", bufs=3) as kgp, \
                 tc.tile_pool(name="it_ps", bufs=2, space="PSUM") as psp:
                for half in range(4):
                    half = half % 2
                    cols = slice(half * 512, (half + 1) * 512)
                    pacc = psp.tile([36, 512], F32, tag="pacc")
                    pairwise_pass(None, t_ya[:, cols], t_q1, pacc[:], kgp, psp, half)

                    # epilogue: y = num/den, r = -|y|^2/h2, hi/lo into t_ya
                    rec = sbp.tile([1, 512], F32, tag="rec")
                    nc.vector.reciprocal(rec[:], pacc[32:33, :])
                    rec3 = sbp.tile([3, 512], F32, tag="rec3")
                    nc.gpsimd.partition_broadcast(rec3[:], rec[:])
                    ynew = sbp.tile([3, 512], F32, tag="ynew")
                    nc.vector.tensor_tensor(out=ynew[:], in0=pacc[0:3, :], in1=rec3[:], op=OP.mult)
                    sq = sbp.tile([3, 512], F32, tag="sq")
                    nc.vector.tensor_tensor(out=sq[:], in0=ynew[:], in1=ynew[:], op=OP.mult)
                    ssum = sbp.tile([3, 512], F32, tag="ssum")
                    nc.gpsimd.partition_all_reduce(ssum[:], sq[:], 3, bass_isa.ReduceOp.add)
                    rf = sbp.tile([1, 512], F32, tag="rf")
                    nc.vector.tensor_scalar_mul(rf[:], ssum[0:1, :], -1.0 / H2)
                    nc.vector.tensor_copy(r_last[:, cols], rf[:])

                    nc.vector.tensor_copy(t_ya[0:3, cols], ynew[:])
                    nc.vector.tensor_copy(t_ya[32:35, cols], ynew[:])
                    yh32 = sbp.tile([3, 512], F32, tag="yh32")
                    nc.vector.tensor_copy(yh32[:], t_ya[0:3, cols])
                    yl16 = sbp.tile([3, 512], F16, tag="yl16")
                    nc.vector.tensor_tensor(out=yl16[:], in0=ynew[:], in1=yh32[:], op=OP.subtract)
                    nc.vector.tensor_copy(t_ya[64:67, cols], yl16[:])
                    nc.vector.tensor_copy(t_ya[96:97, cols], rf[:])
                    rh32 = sbp.tile([1, 512], F32, tag="rh32")
                    nc.vector.tensor_copy(rh32[:], t_ya[96:97, cols])
                    nc.vector.tensor_tensor(out=rl_last[:, cols], in0=rf[:], in1=rh32[:], op=OP.subtract)
                    nc.gpsimd.dma_start(t_ya[97:98, cols], rl_last[:, cols])

        # y output (launch 1 deliverable): y = yh + yl, exact in f32
        with tc.tile_pool(name="yo_sb", bufs=1) as yop:
            yh0a = yop.tile([3, M], F32)
            nc.vector.tensor_copy(yh0a[:], t_ya[0:3, :])
            yl0a = yop.tile([3, M], F32)
            nc.vector.tensor_copy(yl0a[:], t_ya[64:67, :])
            nc.vector.tensor_tensor(out=t_yf[:], in0=yh0a[:], in1=yl0a[:], op=OP.add)
            nc.gpsimd.dma_start(o_y[:], t_yf[:])

        # ==================== final phase (single-launch mode only) ============
        for _frep in range(final_reps if not skip_final else 0):
            with tc.tile_pool(name="fp_one", bufs=1) as one, \
                 tc.tile_pool(name="fp_sb", bufs=2) as sbp, \
                 tc.tile_pool(name="fp_kg", bufs=3) as kgp2, \
                 tc.tile_pool(name="fp_ps", bufs=2, space="PSUM") as psp:

                t_pay = one.tile([3, N], F16)     # allgather payload
                nc.vector.memset(t_pay[:], 0.0)

                # ---- local pieces: y_f32, scaled U hi/lo, c2 hi/lo, payload ----
                yh0 = one.tile([3, M], F32, tag="yh0")
                nc.vector.tensor_copy(yh0[:], t_ya[0:3, :])
                yl0 = one.tile([3, M], F32, tag="yl0")
                nc.vector.tensor_copy(yl0[:], t_ya[64:67, :])
                nc.vector.tensor_tensor(out=t_yf[:], in0=yh0[:], in1=yl0[:], op=OP.add)
                uf = one.tile([3, M], F32, tag="uf")
                nc.vector.tensor_scalar_mul(uf[:], t_yf[:], 2.0 / H2)
                nc.vector.tensor_copy(t_qs2l[0:3, :], uf[:])       # Uh (f16 cast)
                uh32 = one.tile([3, M], F32, tag="uh32")
                nc.vector.tensor_copy(uh32[:], t_qs2l[0:3, :])
                ul16 = one.tile([3, M], F16, tag="ul16")
                nc.vector.tensor_tensor(out=ul16[:], in0=uf[:], in1=uh32[:], op=OP.subtract)
                nc.vector.tensor_copy(t_qs2l[32:35, :], ul16[:])
                nc.vector.tensor_copy(t_qs2l[64:67, :], t_qs2l[0:3, :])
                nc.vector.memset(t_qs2l[96:98, :], 1.0)

                # c2 = r + lnw + LN_C (local), hi/lo
                c2f = one.tile([1, M], F32, tag="c2f")
                nc.vector.tensor_tensor(out=c2f[:], in0=r_last[:], in1=t_lnw[:], op=OP.add)
                c2h = one.tile([1, M], F16, tag="c2h")
                nc.vector.tensor_copy(c2h[:], c2f[:])
                c2h32 = one.tile([1, M], F32, tag="c2h32")
                nc.vector.tensor_copy(c2h32[:], c2h[:])
                c2l = one.tile([1, M], F16, tag="c2l")
                nc.vector.tensor_tensor(out=c2l[:], in0=c2f[:], in1=c2h32[:], op=OP.subtract)

                # payload [3, 8192]: yh | yl | Uh | Ul ; row0: rh | rl | c2h | c2l
                nc.vector.tensor_copy(t_pay[:, 0 * M:1 * M], t_ya[0:3, :])
                nc.vector.tensor_copy(t_pay[:, 1 * M:2 * M], t_ya[64:67, :])
                nc.vector.tensor_copy(t_pay[:, 2 * M:3 * M], t_qs2l[0:3, :])
                nc.vector.tensor_copy(t_pay[:, 3 * M:4 * M], t_qs2l[32:35, :])
                nc.vector.tensor_copy(t_pay[0:1, 4 * M:5 * M], t_ya[96:97, :])
                nc.gpsimd.dma_start(t_pay[0:1, 5 * M:6 * M], t_ya[97:98, :])
                nc.vector.tensor_copy(t_pay[0:1, 6 * M:7 * M], c2h[:])
                nc.vector.tensor_copy(t_pay[0:1, 7 * M:8 * M], c2l[:])

                # THR = -1 - r_i in [p, t] layout
                nc.gpsimd.dma_start(rpt_dram[:], r_last[:])
                nc.gpsimd.dma_start(
                    t_rpt[:],
                    rpt_dram[:].rearrange("one (t p) -> (one p) t", p=128))
                nc.vector.tensor_scalar(out=t_thr[:], in0=t_rpt[:], scalar1=-1.0,
                                        scalar2=-1.0, op0=OP.mult, op1=OP.add)

                # ---- AllGather 1 ----
                nc.gpsimd.dma_start(ag1_in[:], t_pay[:])
                nc.gpsimd.collective_compute(
                    "AllGather", OP.bypass, replica_groups=[list(range(NC))],
                    ins=[ag1_in[:].opt()], outs=[ag1_out[:].opt()])

                # reorder: dst[row, c*M+u] <- ag1_out[c*3+row, col_off+u]
                def reorder(dst_rows, src_row0, src_nrows, col_block):
                    src = ag1_out[:].rearrange("(c r) (b u) -> r c b u", c=NC, u=M)
                    src = src[src_row0:src_row0 + src_nrows, :, col_block, :]
                    dst = dst_rows.rearrange("r (c u) -> r c u", c=NC)
                    nc.gpsimd.dma_start(dst, src)

                reorder(t_rhs2[0:3, :], 0, 3, 0)    # yh_all
                reorder(t_rhs2[32:35, :], 0, 3, 0)  # yh_all again
                reorder(t_rhs2[64:67, :], 0, 3, 1)  # yl_all
                reorder(t_rhs2[96:97, :], 0, 1, 4)  # rh_all
                reorder(t_rhs2[97:98, :], 0, 1, 5)  # rl_all
                reorder(t_qs2[0:3, :], 0, 3, 2)     # Uh_all
                reorder(t_qs2[32:35, :], 0, 3, 3)   # Ul_all
                reorder(t_qs2[64:67, :], 0, 3, 2)   # Uh_all again
                reorder(t_qs2[3:4, :], 0, 1, 6)     # c2h_all
                reorder(t_qs2[35:36, :], 0, 1, 7)   # c2l_all
                nc.vector.memset(t_qs2[96:98, :], 1.0)

                # ---- density pass: same pipeline, ones stationary ----
                dens_loc = one.tile([1, M], F32, tag="dens")
                for half in range(2):
                    cols = slice(half * 512, (half + 1) * 512)
                    pacc = psp.tile([36, 512], F32, tag="pacc")
                    pairwise_pass(t_qs2[:], t_ya[:, cols], None, pacc[:], kgp2, psp, half)
                    nc.vector.tensor_copy(dens_loc[:, cols], pacc[32:33, :])

                # ---- AllGather 2 (density) + broadcast ----
                nc.gpsimd.dma_start(ag3_in[:], dens_loc[:])
                nc.gpsimd.collective_compute(
                    "AllGather", OP.bypass, replica_groups=[list(range(NC))],
                    ins=[ag3_in[:].opt()], outs=[ag3_out[:].opt()])
                nc.gpsimd.dma_start(
                    t_cand[0:1, :], ag3_out[:].rearrange("c u -> (c u)").unsqueeze(0))
                nc.gpsimd.partition_broadcast(t_densb[:], t_cand[0:1, :])

                # ---- mask + argmax pass ([i, j] layout) ----
                for chunk in range(M // 128):
                    ngrp = (16 + GRP - 1) // GRP
                    for g in range(ngrp):
                        njt = min(GRP, 16 - g * GRP)
                        wid = njt * 512
                        pg = psp.tile([128, GRP * 512], F32, tag="pg")
                        for k in range(njt):
                            jt = g * GRP + k
                            nc.tensor.matmul(pg[:, k * 512:(k + 1) * 512],
                                             t_qs2l[:, chunk * 128:(chunk + 1) * 128],
                                             t_rhs2[:, jt * 512:(jt + 1) * 512],
                                             start=True, stop=True)
                        c0 = g * GRP * 512
                        nc.vector.scalar_tensor_tensor(
                            out=t_cand[:, c0:c0 + wid], in0=pg[:, 0:wid],
                            scalar=t_thr[:, chunk:chunk + 1], in1=t_densb[:, c0:c0 + wid],
                            op0=OP.is_ge, op1=OP.mult)
                    nc.vector.max(t_max8[:], t_cand[:])
                    nc.vector.max_index(t_idx8[:], t_max8[:], t_cand[:])
                    nc.vector.tensor_copy(t_idxall[:, chunk:chunk + 1], t_idx8[:, 0:1])

                nc.gpsimd.dma_start(o_y[:], t_yf[:])
                nc.gpsimd.dma_start(o_idx[:], t_idxall[:])


    nc.compile()
    return nc



def build_kernel_l2(reps=1):
    """Launch 2: density pass only (no collectives)."""
    nc = bacc.Bacc("TRN2", target_bir_lowering=False, num_devices=NC)
    d_qs2 = nc.dram_tensor("qs2", [KP, N], F16, kind="ExternalInput")
    d_yaf = nc.dram_tensor("yaf", [KP, M], F16, kind="ExternalInput")
    o_dens = nc.dram_tensor("dens_out", [1, M], F32, kind="ExternalOutput")

    with tile.TileContext(nc) as tc, ExitStack() as ctx:
        per = ctx.enter_context(tc.tile_pool(name="per", bufs=1))
        t_qs2 = per.tile([KP, N], F16)
        t_ya = per.tile([KP, M], F16)
        t_ones = per.tile([128, 36], F16)
        dens_loc = per.tile([1, M], F32)
        nc.gpsimd.dma_start(t_qs2[:], d_qs2[:])
        nc.gpsimd.dma_start(t_ya[:], d_yaf[:])
        nc.vector.memset(t_ones[:], 1.0)

        with tc.For_i(0, reps, 1, hint_engines=(mybir.EngineType.PE,)), \
             tc.tile_pool(name="sb", bufs=3) as kgp, \
             tc.tile_pool(name="ps", bufs=2, space="PSUM") as psp:
            if True:
                for half in range(2):
                    cols = slice(half * 512, (half + 1) * 512)
                    pacc = psp.tile([36, 512], F32, tag="pacc")
                    ngrp = (NCH + GRP - 1) // GRP
                    pending = []

                    def emit_b(g, kg):
                        nch = min(GRP, NCH - g * GRP)
                        for k in range(nch):
                            t = g * GRP + k
                            nc.tensor.matmul(pacc[:], t_ones[:],
                                             kg[:, k * 512:(k + 1) * 512],
                                             start=(t == 0), stop=(t == NCH - 1))

                    for g in range(ngrp):
                        nch = min(GRP, NCH - g * GRP)
                        wid = nch * 512
                        pg = psp.tile([128, GRP * 512], F32, tag="pg")
                        kg = kgp.tile([128, GRP * 512], F16, tag="kg")
                        for k in range(nch):
                            t = g * GRP + k
                            nc.tensor.matmul(pg[:, k * 512:(k + 1) * 512],
                                             t_qs2[:, t * 128:(t + 1) * 128],
                                             t_ya[:, cols], start=True, stop=True)
                        nc.scalar.activation(kg[:, 0:wid], pg[:, 0:wid], AF.Exp,
                                             bias=0.0, scale=1.0)
                        pending.append((g, kg))
                        if len(pending) > 2:
                            emit_b(*pending.pop(0))
                    for p in pending:
                        emit_b(*p)
                    nc.vector.tensor_copy(dens_loc[:, cols], pacc[32:33, :])
            nc.gpsimd.dma_start(o_dens[:], dens_loc[:])
    nc.compile()
    return nc


def build_kernel_l3(reps=1):
    """Launch 3: mask + argmax pass (no collectives)."""
    nc = bacc.Bacc("TRN2", target_bir_lowering=False, num_devices=NC)
    d_qs2l = nc.dram_tensor("qs2l", [KP, M], F16, kind="ExternalInput")
    d_rhs2 = nc.dram_tensor("rhs2", [KP, N], F16, kind="ExternalInput")
    d_dens = nc.dram_tensor("densrow", [1, N], F32, kind="ExternalInput")
    d_thr = nc.dram_tensor("thr", [128, M // 128], F32, kind="ExternalInput")
    o_idx = nc.dram_tensor("idx_out", [128, M // 128], U32, kind="ExternalOutput")

    with tile.TileContext(nc) as tc, ExitStack() as ctx:
        per = ctx.enter_context(tc.tile_pool(name="per", bufs=1))
        t_qs2l = per.tile([KP, M], F16)
        t_rhs2 = per.tile([KP, N], F16)
        t_thr = per.tile([128, M // 128], F32)
        t_densb = per.tile([128, N], F32)
        t_cand = per.tile([128, N], F32)
        t_idxall = per.tile([128, M // 128], U32)
        t_max8 = per.tile([128, 8], F32)
        t_idx8 = per.tile([128, 8], U32)
        nc.gpsimd.dma_start(t_qs2l[:], d_qs2l[:])
        nc.gpsimd.dma_start(t_rhs2[:], d_rhs2[:])
        nc.gpsimd.dma_start(t_thr[:], d_thr[:])
        nc.gpsimd.dma_start(t_densb[0:1, :], d_dens[:])
        nc.gpsimd.partition_broadcast(t_densb[:], t_densb[0:1, :])

        with tc.For_i(0, reps, 1, hint_engines=(mybir.EngineType.PE,)), \
             tc.tile_pool(name="ps", bufs=2, space="PSUM") as psp:
            if True:
                for chunk in range(M // 128):
                    ngrp = (16 + GRP - 1) // GRP
                    for g in range(ngrp):
                        njt = min(GRP, 16 - g * GRP)
                        wid = njt * 512
                        pg = psp.tile([128, GRP * 512], F32, tag="pg")
                        for k in range(njt):
                            jt = g * GRP + k
                            nc.tensor.matmul(pg[:, k * 512:(k + 1) * 512],
                                             t_qs2l[:, chunk * 128:(chunk + 1) * 128],
                                             t_rhs2[:, jt * 512:(jt + 1) * 512],
                                             start=True, stop=True)
                        c0 = g * GRP * 512
                        nc.vector.scalar_tensor_tensor(
                            out=t_cand[:, c0:c0 + wid], in0=pg[:, 0:wid],
                            scalar=t_thr[:, chunk:chunk + 1],
                            in1=t_densb[:, c0:c0 + wid],
                            op0=OP.is_ge, op1=OP.mult)
                    nc.vector.max(t_max8[:], t_cand[:])
                    nc.vector.max_index(t_idx8[:], t_max8[:], t_cand[:])
                    nc.vector.tensor_copy(t_idxall[:, chunk:chunk + 1], t_idx8[:, 0:1])
            nc.gpsimd.dma_start(o_idx[:], t_idxall[:])
    nc.compile()
    return nc


def host_final_prep(y_full, attn_np):
    """Rebuild launch-2/3 operands on the host from gathered final y (f32-exact)."""
    w = np.asarray(attn_np, np.float32)[:, 0]
    lnw = np.log(np.maximum(w, 1e-45)) + LN_C
    y = y_full.astype(np.float32)                 # centered coords [N, 3]
    r = -(y * y).sum(1) / H2
    U = (2.0 / H2) * y.T
    Uh, Ul = _split16(U)
    yh, yl = _split16(y.T)
    rh, rl = _split16(r)
    c2 = r + lnw
    c2h, c2l = _split16(c2)

    qs2 = np.zeros((KP, N), np.float16)           # density stationary (j side)
    qs2[0:3] = Uh; qs2[3] = c2h
    qs2[32:35] = Ul; qs2[35] = c2l
    qs2[64:67] = Uh
    qs2[96] = np.float16(1.0); qs2[97] = np.float16(1.0)

    rhs2 = np.zeros((KP, N), np.float16)          # mask moving (j side)
    rhs2[0:3] = yh; rhs2[32:35] = yh
    rhs2[64:67] = yl
    rhs2[96] = rh; rhs2[97] = rl

    l2_maps, l3_qs2l, l3_thr = [], [], []
    for cidx in range(NC):
        sl = slice(cidx * M, (cidx + 1) * M)
        yaf = np.zeros((KP, M), np.float16)       # density moving (local i side)
        yaf[0:3] = yh[:, sl]; yaf[3] = np.float16(1.0)
        yaf[32:35] = yh[:, sl]; yaf[35] = np.float16(1.0)
        yaf[64:67] = yl[:, sl]
        yaf[96] = rh[sl]; yaf[97] = rl[sl]
        l2_maps.append({"qs2": qs2, "yaf": yaf})

        qs2l = np.zeros((KP, M), np.float16)      # mask stationary (local i side)
        qs2l[0:3] = Uh[:, sl]
        qs2l[32:35] = Ul[:, sl]
        qs2l[64:67] = Uh[:, sl]
        qs2l[96] = np.float16(1.0); qs2l[97] = np.float16(1.0)
        l3_qs2l.append(qs2l)

        thr = (-1.0 - r[sl]).reshape(M // 128, 128).T.astype(np.float32)  # [p, t]
        l3_thr.append(thr)
    return l2_maps, l3_qs2l, l3_thr, rhs2


def host_prep(q_np, attn_np):
    q = np.asarray(q_np, np.float32) - 0.5
    w = np.asarray(attn_np, np.float32)[:, 0]
    lnw = np.log(np.maximum(w, 1e-45)) + LN_C

    QS = (2.0 / H2) * q.T                    # [3, N]
    Qh, Ql = _split16(QS)
    c = -(q * q).sum(1) / H2 + lnw           # [N]
    ch, cl = _split16(c)

    qs = np.zeros((KP, N), np.float16)
    qs[0:3] = Qh; qs[3] = ch
    qs[32:35] = Ql; qs[35] = cl
    qs[64:67] = Qh
    qs[96] = np.float16(1.0); qs[97] = np.float16(1.0)

    q1w = np.zeros((N, 36), np.float32)   # [q, 0...,1@32,...] -> den lands at partition 32
    q1w[:, 0:3] = q
    q1w[:, 32] = 1.0
    q1 = q1w.reshape(NCH, 128, 36).transpose(1, 0, 2).reshape(128, 36 * NCH).astype(np.float16)

    in_maps = []
    for cidx in range(NC):
        sl = slice(cidx * M, (cidx + 1) * M)
        yloc = q[sl]
        r0 = -(yloc * yloc).sum(1) / H2
        yh, yl = _split16(yloc.T)
        rh, rl = _split16(r0)
        ya0 = np.zeros((KP, M), np.float16)
        ya0[0:3] = yh; ya0[3] = np.float16(1.0)
        ya0[32:35] = yh; ya0[35] = np.float16(1.0)
        ya0[64:67] = yl
        ya0[96] = rh; ya0[97] = rl
        in_maps.append({
            "qs": qs, "q1": q1, "ya0": ya0,
            "lnwloc": lnw[sl].reshape(1, M).astype(np.float32),
        })
    return in_maps


def postprocess(results):
    y_full = np.concatenate([r["y_out"].T for r in results], axis=0)  # [N, 3]
    idx = np.concatenate(
        [r["idx_out"].T.reshape(-1) for r in results])                # [N] (t*128+p order)
    return (y_full[idx.astype(np.int64)] + 0.5).astype(np.float32)


def kernel(q, attn):
    if "l1" not in _cache:
        _cache["l1"] = build_kernel(skip_final=True)
        _cache["l2"] = build_kernel_l2()
        _cache["l3"] = build_kernel_l3()
    cores = list(range(NC))

    in_maps = host_prep(q, attn)
    res1 = bass_utils.run_bass_kernel_spmd(_cache["l1"], in_maps, core_ids=cores)
    y_full = np.concatenate([r["y_out"].T for r in res1.results], axis=0)  # [N,3] centered

    l2_maps, l3_qs2l, l3_thr, rhs2 = host_final_prep(y_full, attn)
    res2 = bass_utils.run_bass_kernel_spmd(_cache["l2"], l2_maps, core_ids=cores)
    dens = np.concatenate([r["dens_out"][0] for r in res2.results]).reshape(1, N)

    l3_maps = [{"qs2l": l3_qs2l[c], "rhs2": rhs2,
                "densrow": dens.astype(np.float32), "thr": l3_thr[c]} for c in cores]
    res3 = bass_utils.run_bass_kernel_spmd(_cache["l3"], l3_maps, core_ids=cores)

    idx = np.concatenate([r["idx_out"].T.reshape(-1) for r in res3.results])
    return (y_full[idx.astype(np.int64)] + 0.5).astype(np.float32)


if __name__ == "__main__":
    import reference as refmod
    inputs = {k: np.asarray(v) for k, v in refmod.setup_inputs().items()}
    expected = np.asarray(refmod.reference(**inputs))
    out = kernel(**inputs)
    rel = np.linalg.norm(out - expected) / np.linalg.norm(expected)
    print("Relative error:", rel)



# revision 4
# speedup vs baseline: 1.5839x; 1.5839x over previous
"""MeanShiftClusterer Trainium2 Bass kernel (8 NeuronCores, SPMD).

Algorithm (reference: 10 mean-shift iterations + mode snap):
  iter:  K[i,j] = exp(-(|y_i - q_j|^2)/h^2) * w_j ; y <- (K@q)/rowsum(K)
  final: density[i] = rowsum(K(y,y)*w) ; snap each i to argmax_j density[j]
         over j within one bandwidth.

Device mapping per core (1024 query rows each, all 8192 sources):
  - Pairwise exponent via ONE K~98 fp16 matmul: hi/lo split rows at
    partition offsets {0,32,64,96} (engine partition-alignment), constant
    and c_j terms folded into spare rows -> plain Exp on ACT, ~1.4e-4 abs err.
  - ACT exp (fp16 out, scaled by 2^10 for fp16 headroom) -> B matmul
    ([q,1] fp16 stationary) accumulates numerator+denominator in PSUM.
  - Final: AllGather 8KB fp16 payload, density pass (same pipeline,
    ones stationary), AllGather density, mask+argmax via STT + max_index.
Host: trivial layout prep (centering, hi/lo splits) + final y[idx] gather.
"""
import sys
import numpy as np

sys.path.insert(0, '/opt/trn_rl_repo')

import concourse.bacc as bacc
import concourse.tile as tile
import concourse.mybir as mybir
import concourse.bass_isa as bass_isa
from concourse import bass_utils
from contextlib import ExitStack

dt = mybir.dt
F32, F16, U32 = dt.float32, dt.float16, dt.uint32
AF = mybir.ActivationFunctionType
OP = mybir.AluOpType

N = 8192
NC = 8
M = N // NC              # 1024 local query rows per core
H2 = 0.05 * 0.05
LN_C = float(np.log(2.0 ** 10))
N_ITERS = 10
NCH = N // 128           # 64 j-chunks
GRP = 3                  # j-chunks per ACT group (3 banks * 2 bufs + 2 acc = 8)
KP = 98                  # padded contraction dim (rows at 0..2,3,32..34,35,64..66,96,97)

_cache = {}


def _split16(x):
    h = x.astype(np.float16)
    l = (x.astype(np.float32) - h.astype(np.float32)).astype(np.float16)
    return h, l


def build_kernel(n_iters=N_ITERS, skip_final=False, final_reps=1):
    nc = bacc.Bacc("TRN2", target_bir_lowering=False, num_devices=NC)

    d_qs = nc.dram_tensor("qs", [KP, N], F16, kind="ExternalInput")
    d_q1 = nc.dram_tensor("q1", [128, 36 * NCH], F16, kind="ExternalInput")
    d_ya0 = nc.dram_tensor("ya0", [KP, M], F16, kind="ExternalInput")
    d_lnw = nc.dram_tensor("lnwloc", [1, M], F32, kind="ExternalInput")  # ln w + LN_C, local slice

    o_y = nc.dram_tensor("y_out", [3, M], F32, kind="ExternalOutput")
    o_idx = nc.dram_tensor("idx_out", [128, M // 128], U32, kind="ExternalOutput")

    with tile.TileContext(nc) as tc, ExitStack() as ctx:
        per = ctx.enter_context(tc.tile_pool(name="per", bufs=1))         # persistents
        dram = ctx.enter_context(tc.tile_pool(name="dram", bufs=1, space="DRAM"))

        # ---- persistent SBUF ----
        t_qs = per.tile([KP, N], F16)
        t_q1 = per.tile([128, 36 * NCH], F16)
        t_ya = per.tile([KP, M], F16)
        t_lnw = per.tile([1, M], F32)
        t_ones4 = per.tile([128, 36], F16)
        r_last = per.tile([1, M], F32)
        rl_last = per.tile([1, M], F16)
        t_qs2 = per.tile([KP, N], F16)    # density-pass stationary (j side, scaled)
        t_rhs2 = per.tile([KP, N], F16)   # mask-pass moving (j side, unscaled)
        t_qs2l = per.tile([KP, M], F16)   # mask-pass stationary (i side, scaled)
        t_densb = per.tile([128, N], F32)
        t_cand = per.tile([128, N], F32)
        t_rpt = per.tile([128, M // 128], F32)
        t_thr = per.tile([128, M // 128], F32)
        t_yf = per.tile([3, M], F32)
        t_idxall = per.tile([128, M // 128], U32)
        t_max8 = per.tile([128, 8], F32)
        t_idx8 = per.tile([128, 8], U32)

        # DRAM bounce buffers for collectives
        ag1_in = dram.tile([3, N], F16)
        ag1_out = dram.tile([3 * NC, N], F16)
        ag3_in = dram.tile([1, M], F32)
        ag3_out = dram.tile([NC, M], F32)
        rpt_dram = dram.tile([1, M], F32)

        nc.gpsimd.dma_start(t_qs[:], d_qs[:])
        nc.gpsimd.dma_start(t_q1[:], d_q1[:])
        nc.gpsimd.dma_start(t_lnw[:], d_lnw[:])
        nc.vector.memset(t_ones4[:], 1.0)
        nc.vector.memset(t_qs2[:], 0.0)
        nc.vector.memset(t_rhs2[:], 0.0)
        nc.vector.memset(t_qs2l[:], 0.0)

        nc.gpsimd.dma_start(t_ya[:], d_ya0[:])

        def pairwise_pass(lhs_a, rhs_ya, lhs_b, acc_into, kg_pool, ps_pool, half):
            """64-chunk j loop: A matmul -> exp -> B matmul accumulate.

            lhs_a: [KP, N] stationary (exponent terms), rhs_ya: [KP, 512] moving,
            lhs_b: [128, >=4] B stationary source, acc_into: psum [4, 512].
            """
            ngrp = (NCH + GRP - 1) // GRP

            def emit_b(g, kg):
                nch = min(GRP, NCH - g * GRP)
                for k in range(nch):
                    t = g * GRP + k
                    nc.tensor.matmul(acc_into,
                                     lhs_b[:, t * 36:(t + 1) * 36] if lhs_b is not None
                                     else t_ones4[:],
                                     kg[:, k * 512:(k + 1) * 512],
                                     start=(t == 0), stop=(t == NCH - 1))

            # depth-2 software pipeline: emit A(g)+ACT(g), then B(g-2) —
            # B(g-2)'s ACT finished two groups ago, so the in-order PE queue
            # never stalls on ACT.
            pending = []
            for g in range(ngrp):
                nch = min(GRP, NCH - g * GRP)
                wid = nch * 512
                pg = ps_pool.tile([128, GRP * 512], F32, tag="pg")
                kg = kg_pool.tile([128, GRP * 512], F16, tag="kg")
                for k in range(nch):
                    t = g * GRP + k
                    nc.tensor.matmul(pg[:, k * 512:(k + 1) * 512],
                                     t_qs[:, t * 128:(t + 1) * 128] if lhs_a is None
                                     else lhs_a[:, t * 128:(t + 1) * 128],
                                     rhs_ya, start=True, stop=True)
                nc.scalar.activation(kg[:, 0:wid], pg[:, 0:wid], AF.Exp,
                                     bias=0.0, scale=1.0)
                pending.append((g, kg))
                if len(pending) > 2:
                    emit_b(*pending.pop(0))
            for p in pending:
                emit_b(*p)

        # ==================== mean-shift iterations ====================
        assert n_iters % 2 == 0
        with tc.For_i(0, n_iters // 2, 1, hint_engines=(mybir.EngineType.PE,)):
            with tc.tile_pool(name="it_sb", bufs=2) as sbp, \
                 tc.tile_pool(name="it_kg", bufs=3) as kgp, \
                 tc.tile_pool(name="it_ps", bufs=2, space="PSUM") as psp:
                for half in range(4):
                    half = half % 2
                    cols = slice(half * 512, (half + 1) * 512)
                    pacc = psp.tile([36, 512], F32, tag="pacc")
                    pairwise_pass(None, t_ya[:, cols], t_q1, pacc[:], kgp, psp, half)

                    # epilogue: y = num/den, r = -|y|^2/h2, hi/lo into t_ya
                    rec = sbp.tile([1, 512], F32, tag="rec")
                    nc.vector.reciprocal(rec[:], pacc[32:33, :])
                    rec3 = sbp.tile([3, 512], F32, tag="rec3")
                    nc.gpsimd.partition_broadcast(rec3[:], rec[:])
                    ynew = sbp.tile([3, 512], F32, tag="ynew")
                    nc.vector.tensor_tensor(out=ynew[:], in0=pacc[0:3, :], in1=rec3[:], op=OP.mult)
                    sq = sbp.tile([3, 512], F32, tag="sq")
                    nc.vector.tensor_tensor(out=sq[:], in0=ynew[:], in1=ynew[:], op=OP.mult)
                    ssum = sbp.tile([3, 512], F32, tag="ssum")
                    nc.gpsimd.partition_all_reduce(ssum[:], sq[:], 3, bass_isa.ReduceOp.add)
                    rf = sbp.tile([1, 512], F32, tag="rf")
                    nc.vector.tensor_scalar_mul(rf[:], ssum[0:1, :], -1.0 / H2)
                    nc.vector.tensor_copy(r_last[:, cols], rf[:])

                    nc.vector.tensor_copy(t_ya[0:3, cols], ynew[:])
                    nc.vector.tensor_copy(t_ya[32:35, cols], ynew[:])
                    yh32 = sbp.tile([3, 512], F32, tag="yh32")
                    nc.vector.tensor_copy(yh32[:], t_ya[0:3, cols])
                    yl16 = sbp.tile([3, 512], F16, tag="yl16")
                    nc.vector.tensor_tensor(out=yl16[:], in0=ynew[:], in1=yh32[:], op=OP.subtract)
                    nc.vector.tensor_copy(t_ya[64:67, cols], yl16[:])
                    nc.vector.tensor_copy(t_ya[96:97, cols], rf[:])
                    rh32 = sbp.tile([1, 512], F32, tag="rh32")
                    nc.vector.tensor_copy(rh32[:], t_ya[96:97, cols])
                    nc.vector.tensor_tensor(out=rl_last[:, cols], in0=rf[:], in1=rh32[:], op=OP.subtract)
                    nc.gpsimd.dma_start(t_ya[97:98, cols], rl_last[:, cols])

        # y output (launch 1 deliverable): y = yh + yl, exact in f32
        with tc.tile_pool(name="yo_sb", bufs=1) as yop:
            yh0a = yop.tile([3, M], F32)
            nc.vector.tensor_copy(yh0a[:], t_ya[0:3, :])
            yl0a = yop.tile([3, M], F32)
            nc.vector.tensor_copy(yl0a[:], t_ya[64:67, :])
            nc.vector.tensor_tensor(out=t_yf[:], in0=yh0a[:], in1=yl0a[:], op=OP.add)
            nc.gpsimd.dma_start(o_y[:], t_yf[:])

        # ==================== final phase (single-launch mode only) ============
        for _frep in range(final_reps if not skip_final else 0):
            with tc.tile_pool(name="fp_one", bufs=1) as one, \
                 tc.tile_pool(name="fp_sb", bufs=2) as sbp, \
                 tc.tile_pool(name="fp_kg", bufs=3) as kgp2, \
                 tc.tile_pool(name="fp_ps", bufs=2, space="PSUM") as psp:

                t_pay = one.tile([3, N], F16)     # allgather payload
                nc.vector.memset(t_pay[:], 0.0)

                # ---- local pieces: y_f32, scaled U hi/lo, c2 hi/lo, payload ----
                yh0 = one.tile([3, M], F32, tag="yh0")
                nc.vector.tensor_copy(yh0[:], t_ya[0:3, :])
                yl0 = one.tile([3, M], F32, tag="yl0")
                nc.vector.tensor_copy(yl0[:], t_ya[64:67, :])
                nc.vector.tensor_tensor(out=t_yf[:], in0=yh0[:], in1=yl0[:], op=OP.add)
                uf = one.tile([3, M], F32, tag="uf")
                nc.vector.tensor_scalar_mul(uf[:], t_yf[:], 2.0 / H2)
                nc.vector.tensor_copy(t_qs2l[0:3, :], uf[:])       # Uh (f16 cast)
                uh32 = one.tile([3, M], F32, tag="uh32")
                nc.vector.tensor_copy(uh32[:], t_qs2l[0:3, :])
                ul16 = one.tile([3, M], F16, tag="ul16")
                nc.vector.tensor_tensor(out=ul16[:], in0=uf[:], in1=uh32[:], op=OP.subtract)
                nc.vector.tensor_copy(t_qs2l[32:35, :], ul16[:])
                nc.vector.tensor_copy(t_qs2l[64:67, :], t_qs2l[0:3, :])
                nc.vector.memset(t_qs2l[96:98, :], 1.0)

                # c2 = r + lnw + LN_C (local), hi/lo
                c2f = one.tile([1, M], F32, tag="c2f")
                nc.vector.tensor_tensor(out=c2f[:], in0=r_last[:], in1=t_lnw[:], op=OP.add)
                c2h = one.tile([1, M], F16, tag="c2h")
                nc.vector.tensor_copy(c2h[:], c2f[:])
                c2h32 = one.tile([1, M], F32, tag="c2h32")
                nc.vector.tensor_copy(c2h32[:], c2h[:])
                c2l = one.tile([1, M], F16, tag="c2l")
                nc.vector.tensor_tensor(out=c2l[:], in0=c2f[:], in1=c2h32[:], op=OP.subtract)

                # payload [3, 8192]: yh | yl | Uh | Ul ; row0: rh | rl | c2h | c2l
                nc.vector.tensor_copy(t_pay[:, 0 * M:1 * M], t_ya[0:3, :])
                nc.vector.tensor_copy(t_pay[:, 1 * M:2 * M], t_ya[64:67, :])
                nc.vector.tensor_copy(t_pay[:, 2 * M:3 * M], t_qs2l[0:3, :])
                nc.vector.tensor_copy(t_pay[:, 3 * M:4 * M], t_qs2l[32:35, :])
                nc.vector.tensor_copy(t_pay[0:1, 4 * M:5 * M], t_ya[96:97, :])
                nc.gpsimd.dma_start(t_pay[0:1, 5 * M:6 * M], t_ya[97:98, :])
                nc.vector.tensor_copy(t_pay[0:1, 6 * M:7 * M], c2h[:])
                nc.vector.tensor_copy(t_pay[0:1, 7 * M:8 * M], c2l[:])

                # THR = -1 - r_i in [p, t] layout
                nc.gpsimd.dma_start(rpt_dram[:], r_last[:])
                nc.gpsimd.dma_start(
                    t_rpt[:],
                    rpt_dram[:].rearrange("one (t p) -> (one p) t", p=128))
                nc.vector.tensor_scalar(out=t_thr[:], in0=t_rpt[:], scalar1=-1.0,
                                        scalar2=-1.0, op0=OP.mult, op1=OP.add)

                # ---- AllGather 1 ----
                nc.gpsimd.dma_start(ag1_in[:], t_pay[:])
                nc.gpsimd.collective_compute(
                    "AllGather", OP.bypass, replica_groups=[list(range(NC))],
                    ins=[ag1_in[:].opt()], outs=[ag1_out[:].opt()])

                # reorder: dst[row, c*M+u] <- ag1_out[c*3+row, col_off+u]
                def reorder(dst_rows, src_row0, src_nrows, col_block):
                    src = ag1_out[:].rearrange("(c r) (b u) -> r c b u", c=NC, u=M)
                    src = src[src_row0:src_row0 + src_nrows, :, col_block, :]
                    dst = dst_rows.rearrange("r (c u) -> r c u", c=NC)
                    nc.gpsimd.dma_start(dst, src)

                reorder(t_rhs2[0:3, :], 0, 3, 0)    # yh_all
                reorder(t_rhs2[32:35, :], 0, 3, 0)  # yh_all again
                reorder(t_rhs2[64:67, :], 0, 3, 1)  # yl_all
                reorder(t_rhs2[96:97, :], 0, 1, 4)  # rh_all
                reorder(t_rhs2[97:98, :], 0, 1, 5)  # rl_all
                reorder(t_qs2[0:3, :], 0, 3, 2)     # Uh_all
                reorder(t_qs2[32:35, :], 0, 3, 3)   # Ul_all
                reorder(t_qs2[64:67, :], 0, 3, 2)   # Uh_all again
                reorder(t_qs2[3:4, :], 0, 1, 6)     # c2h_all
                reorder(t_qs2[35:36, :], 0, 1, 7)   # c2l_all
                nc.vector.memset(t_qs2[96:98, :], 1.0)

                # ---- density pass: same pipeline, ones stationary ----
                dens_loc = one.tile([1, M], F32, tag="dens")
                for half in range(2):
                    cols = slice(half * 512, (half + 1) * 512)
                    pacc = psp.tile([36, 512], F32, tag="pacc")
                    pairwise_pass(t_qs2[:], t_ya[:, cols], None, pacc[:], kgp2, psp, half)
                    nc.vector.tensor_copy(dens_loc[:, cols], pacc[32:33, :])

                # ---- AllGather 2 (density) + broadcast ----
                nc.gpsimd.dma_start(ag3_in[:], dens_loc[:])
                nc.gpsimd.collective_compute(
                    "AllGather", OP.bypass, replica_groups=[list(range(NC))],
                    ins=[ag3_in[:].opt()], outs=[ag3_out[:].opt()])
                nc.gpsimd.dma_start(
                    t_cand[0:1, :], ag3_out[:].rearrange("c u -> (c u)").unsqueeze(0))
                nc.gpsimd.partition_broadcast(t_densb[:], t_cand[0:1, :])

                # ---- mask + argmax pass ([i, j] layout) ----
                for chunk in range(M // 128):
                    ngrp = (16 + GRP - 1) // GRP
                    for g in range(ngrp):
                        njt = min(GRP, 16 - g * GRP)
                        wid = njt * 512
                        pg = psp.tile([128, GRP * 512], F32, tag="pg")
                        for k in range(njt):
                            jt = g * GRP + k
                            nc.tensor.matmul(pg[:, k * 512:(k + 1) * 512],
                                             t_qs2l[:, chunk * 128:(chunk + 1) * 128],
                                             t_rhs2[:, jt * 512:(jt + 1) * 512],
                                             start=True, stop=True)
                        c0 = g * GRP * 512
                        nc.vector.scalar_tensor_tensor(
                            out=t_cand[:, c0:c0 + wid], in0=pg[:, 0:wid],
                            scalar=t_thr[:, chunk:chunk + 1], in1=t_densb[:, c0:c0 + wid],
                            op0=OP.is_ge, op1=OP.mult)
                    nc.vector.max(t_max8[:], t_cand[:])
                    nc.vector.max_index(t_idx8[:], t_max8[:], t_cand[:])
                    nc.vector.tensor_copy(t_idxall[:, chunk:chunk + 1], t_idx8[:, 0:1])

                nc.gpsimd.dma_start(o_y[:], t_yf[:])
                nc.gpsimd.dma_start(o_idx[:], t_idxall[:])


    nc.compile()
    return nc




# ==================== final phase (redesigned) ====================
#
# After L1 the host holds the converged positions y.  Everything below runs
# in Morton-order of y ("sorted frame") so that spatially-near queries are
# contiguous and the per-chunk schedules are dense.
#
#   L2  density pass: scheduled j-chunks only (bbox cutoff 0.15), staged
#       stationaries as data -> one SPMD program for all cores.
#   L3  mode snap: [j,i] layout; pg = 1 - d2/h2 via one matmul, ACT copies
#       PSUM->fp16, DVE (pc>=0)*qr with per-partition rank scalars, 4-way
#       fold-max over chunks, PE transpose, top-8 max_index.  Host finishes
#       with an exact argmax over ~O(100) candidates per query.

C2_CUT = 0.15            # density exclusion radius (exp(-9) ~ 1e-4)
C3_CUT = 0.055           # mask candidate radius (h + bbox slop)


def build_kernel_l2(s2max, reps=1):
    """Density pass with staged (scheduled) stationary chunks."""
    nc = bacc.Bacc("TRN2", target_bir_lowering=False, num_devices=NC)
    d_qs2 = nc.dram_tensor("qs2s", [KP, 2 * s2max * 128], F16, kind="ExternalInput")
    d_yaf = nc.dram_tensor("yaf", [KP, M], F16, kind="ExternalInput")
    o_dens = nc.dram_tensor("dens_out", [1, M], F32, kind="ExternalOutput")

    with tile.TileContext(nc) as tc, ExitStack() as ctx:
        per = ctx.enter_context(tc.tile_pool(name="per", bufs=1))
        t_qs2 = per.tile([KP, 2 * s2max * 128], F16)
        t_ya = per.tile([KP, M], F16)
        t_ones = per.tile([128, 36], F16)
        dens_loc = per.tile([1, M], F32)
        nc.gpsimd.dma_start(t_qs2[:], d_qs2[:])
        nc.gpsimd.dma_start(t_ya[:], d_yaf[:])
        nc.vector.memset(t_ones[:], 1.0)

        with tc.For_i(0, reps, 1, hint_engines=(mybir.EngineType.PE,)), \
             tc.tile_pool(name="sb", bufs=3) as kgp, \
             tc.tile_pool(name="ps", bufs=2, space="PSUM") as psp:
            for half in range(2):
                cols = slice(half * 512, (half + 1) * 512)
                base = half * s2max * 128
                pacc = psp.tile([36, 512], F32, tag="pacc")
                ngrp = (s2max + GRP - 1) // GRP
                pending = []

                def emit_b(g, kg):
                    nch = min(GRP, s2max - g * GRP)
                    for k in range(nch):
                        t = g * GRP + k
                        nc.tensor.matmul(pacc[:], t_ones[:],
                                         kg[:, k * 512:(k + 1) * 512],
                                         start=(t == 0), stop=(t == s2max - 1))

                for g in range(ngrp):
                    nch = min(GRP, s2max - g * GRP)
                    wid = nch * 512
                    pg = psp.tile([128, GRP * 512], F32, tag="pg")
                    kg = kgp.tile([128, GRP * 512], F16, tag="kg")
                    for k in range(nch):
                        t = g * GRP + k
                        nc.tensor.matmul(pg[:, k * 512:(k + 1) * 512],
                                         t_qs2[:, base + t * 128:base + (t + 1) * 128],
                                         t_ya[:, cols], start=True, stop=True)
                    nc.scalar.activation(kg[:, 0:wid], pg[:, 0:wid], AF.Exp,
                                         bias=0.0, scale=1.0)
                    pending.append((g, kg))
                    if len(pending) > 2:
                        emit_b(*pending.pop(0))
                for p in pending:
                    emit_b(*p)
                nc.vector.tensor_copy(dens_loc[:, cols], pacc[32:33, :])
            nc.gpsimd.dma_start(o_dens[:], dens_loc[:])
    nc.compile()
    return nc


def build_kernel_l3(s3max, reps=1):
    """Mode-snap pass: [j,i] layout, staged j-chunk stationaries, top-8 out."""
    nc = bacc.Bacc("TRN2", target_bir_lowering=False, num_devices=NC)
    d_st = nc.dram_tensor("st3", [KP, 2 * s3max * 128], F16, kind="ExternalInput")
    d_ya = nc.dram_tensor("ya3", [KP, M], F16, kind="ExternalInput")
    d_qr = nc.dram_tensor("qr3", [128, 2 * s3max], F32, kind="ExternalInput")
    d_id = nc.dram_tensor("ident", [128, 128], F16, kind="ExternalInput")
    o_idx = nc.dram_tensor("idx8", [128, 64], U32, kind="ExternalOutput")

    with tile.TileContext(nc) as tc, ExitStack() as ctx:
        per = ctx.enter_context(tc.tile_pool(name="per", bufs=1))
        t_st = per.tile([KP, 2 * s3max * 128], F16)
        t_ya = per.tile([KP, M], F16)
        t_qr = per.tile([128, 2 * s3max], F32)
        t_id = per.tile([128, 128], F16)
        t_out = per.tile([128, 64], U32)
        nc.gpsimd.dma_start(t_st[:], d_st[:])
        nc.gpsimd.dma_start(t_ya[:], d_ya[:])
        nc.gpsimd.dma_start(t_qr[:], d_qr[:])
        nc.gpsimd.dma_start(t_id[:], d_id[:])

        with tc.For_i(0, reps, 1, hint_engines=(mybir.EngineType.PE,)), \
             tc.tile_pool(name="sb", bufs=3) as sbp, \
             tc.tile_pool(name="fold", bufs=1) as fop, \
             tc.tile_pool(name="ps", bufs=3, space="PSUM") as psp, \
             tc.tile_pool(name="pst", bufs=2, space="PSUM") as pst:
            for half in range(2):
                cols = slice(half * 512, (half + 1) * 512)
                base = half * s3max * 128
                folds = []
                for k in range(4):
                    f = fop.tile([128, 512], F16, tag=f"fold{half}_{k}")
                    nc.vector.memset(f[:], 0.0)
                    folds.append(f)
                for t in range(s3max):
                    pg = psp.tile([128, 512], F32, tag="pg")
                    nc.tensor.matmul(pg[:],
                                     t_st[:, base + t * 128:base + (t + 1) * 128],
                                     t_ya[:, cols], start=True, stop=True)
                    pc = sbp.tile([128, 512], F16, tag="pc")
                    nc.scalar.activation(pc[:], pg[:], AF.Copy, bias=0.0, scale=1.0)
                    cand = sbp.tile([128, 512], F16, tag="cand")
                    nc.vector.tensor_scalar(out=cand[:], in0=pc[:],
                                            scalar1=0.0,
                                            scalar2=t_qr[:, half * s3max + t:half * s3max + t + 1],
                                            op0=OP.is_ge, op1=OP.mult)
                    nc.vector.tensor_tensor(out=folds[t % 4][:], in0=folds[t % 4][:],
                                            in1=cand[:], op=OP.max)
                nc.vector.tensor_tensor(out=folds[0][:], in0=folds[0][:],
                                        in1=folds[1][:], op=OP.max)
                nc.vector.tensor_tensor(out=folds[2][:], in0=folds[2][:],
                                        in1=folds[3][:], op=OP.max)
                nc.vector.tensor_tensor(out=folds[0][:], in0=folds[0][:],
                                        in1=folds[2][:], op=OP.max)
                for b in range(4):
                    tr = pst.tile([128, 128], F16, tag="tr")
                    nc.tensor.matmul(tr[:], folds[0][:, b * 128:(b + 1) * 128],
                                     t_id[:], start=True, stop=True,
                                     is_transpose=True)
                    m8 = sbp.tile([128, 8], F16, tag="m8")
                    i8 = sbp.tile([128, 8], U32, tag="i8")
                    nc.vector.max(m8[:], tr[:])
                    nc.vector.max_index(i8[:], m8[:], tr[:])
                    nc.vector.tensor_copy(
                        t_out[:, (half * 4 + b) * 8:(half * 4 + b) * 8 + 8], i8[:])
            nc.gpsimd.dma_start(o_idx[:], t_out[:])
    nc.compile()
    return nc


def host_prep(q_np, attn_np):
    q = np.asarray(q_np, np.float32) - 0.5
    w = np.asarray(attn_np, np.float32)[:, 0]
    lnw = np.log(np.maximum(w, 1e-45)) + LN_C

    QS = (2.0 / H2) * q.T                    # [3, N]
    Qh, Ql = _split16(QS)
    c = -(q * q).sum(1) / H2 + lnw           # [N]
    ch, cl = _split16(c)

    qs = np.zeros((KP, N), np.float16)
    qs[0:3] = Qh; qs[3] = ch
    qs[32:35] = Ql; qs[35] = cl
    qs[64:67] = Qh
    qs[96] = np.float16(1.0); qs[97] = np.float16(1.0)

    q1w = np.zeros((N, 36), np.float32)   # [q, 0...,1@32,...] -> den lands at partition 32
    q1w[:, 0:3] = q
    q1w[:, 32] = 1.0
    q1 = q1w.reshape(NCH, 128, 36).transpose(1, 0, 2).reshape(128, 36 * NCH).astype(np.float16)

    in_maps = []
    for cidx in range(NC):
        sl = slice(cidx * M, (cidx + 1) * M)
        yloc = q[sl]
        r0 = -(yloc * yloc).sum(1) / H2
        yh, yl = _split16(yloc.T)
        rh, rl = _split16(r0)
        ya0 = np.zeros((KP, M), np.float16)
        ya0[0:3] = yh; ya0[3] = np.float16(1.0)
        ya0[32:35] = yh; ya0[35] = np.float16(1.0)
        ya0[64:67] = yl
        ya0[96] = rh; ya0[97] = rl
        in_maps.append({
            "qs": qs, "q1": q1, "ya0": ya0,
            "lnwloc": lnw[sl].reshape(1, M).astype(np.float32),
        })
    return in_maps




# ==================== host-side final phase ====================

def _morton(p, bits=10):
    qi = np.clip((p * (1 << bits)).astype(np.int64), 0, (1 << bits) - 1)
    code = np.zeros(len(p), np.int64)
    for b in range(bits):
        for d in range(3):
            code |= ((qi[:, d] >> b) & 1) << (3 * b + d)
    return code


def _bboxes(pts, bs):
    r = pts.reshape(-1, bs, 3)
    return r.min(1), r.max(1)


def _boxdist(lo1, hi1, lo2, hi2):
    d = np.maximum(np.maximum(lo1[:, None] - hi2[None, :],
                              lo2[None, :] - hi1[:, None]), 0.0)
    return np.sqrt((d * d).sum(-1))


def _stat_embed(y, crow):
    """[KP, n] stationary embedding: U rows + per-point constant row."""
    n = len(y)
    U = (2.0 / H2) * y.T
    Uh, Ul = _split16(U)
    ch, cl = _split16(crow)
    st = np.zeros((KP, n), np.float16)
    st[0:3] = Uh; st[3] = ch
    st[32:35] = Ul; st[35] = cl
    st[64:67] = Uh
    st[96] = np.float16(1.0); st[97] = np.float16(1.0)
    return st


def _mov_embed(y):
    """[KP, n] moving embedding: y rows + r rows."""
    n = len(y)
    r = -(y * y).sum(1) / H2
    yh, yl = _split16(y.T)
    rh, rl = _split16(r)
    mv = np.zeros((KP, n), np.float16)
    mv[0:3] = yh; mv[3] = np.float16(1.0)
    mv[32:35] = yh; mv[35] = np.float16(1.0)
    mv[64:67] = yl
    mv[96] = rh; mv[97] = rl
    return mv


def final_phase_prep(y_full, attn_np):
    """Sort by Morton(y), build L2/L3 schedules + staged inputs."""
    w = np.asarray(attn_np, np.float32)[:, 0]
    y = np.asarray(y_full, np.float64)            # centered coords
    perm2 = np.argsort(_morton(np.clip(y + 0.5, 0.0, 0.999999)))
    ys = y[perm2]; ws = w[perm2]

    ilo, ihi = _bboxes(ys, 512)
    jlo, jhi = _bboxes(ys, 128)
    D = _boxdist(ilo, ihi, jlo, jhi)              # [16 halves, 64 chunks]
    sched2 = [np.nonzero(D[b] <= C2_CUT)[0] for b in range(16)]
    sched3 = [np.nonzero(D[b] <= C3_CUT)[0] for b in range(16)]
    s2max = max(len(s) for s in sched2)
    s3max = max(len(s) for s in sched3)

    lnw = np.log(np.maximum(ws, 1e-45)) + LN_C
    r_j = -(ys * ys).sum(1) / H2
    st2 = _stat_embed(ys, r_j + lnw)              # density stationary
    st3 = _stat_embed(ys, r_j + 1.0)              # mask stationary (pg = 1 - d2/h2)
    mv = _mov_embed(ys)

    l2_maps, l3_st, l3_qr = [], [], []
    for c in range(NC):
        q2 = np.zeros((KP, 2 * s2max * 128), np.float16)
        q2[3] = np.float16(-60000.0)              # padding: exp(-huge) = 0
        q3 = np.zeros((KP, 2 * s3max * 128), np.float16)
        q3[3] = np.float16(-60000.0)              # padding: pg = -huge
        qr = np.zeros((128, 2 * s3max), np.float32)
        for h in range(2):
            b = c * 2 + h
            for s, ch in enumerate(sched2[b]):
                q2[:, (h * s2max + s) * 128:(h * s2max + s + 1) * 128] = \
                    st2[:, ch * 128:(ch + 1) * 128]
            for s, ch in enumerate(sched3[b]):
                q3[:, (h * s3max + s) * 128:(h * s3max + s + 1) * 128] = \
                    st3[:, ch * 128:(ch + 1) * 128]
        l2_maps.append({"qs2s": q2, "yaf": np.ascontiguousarray(mv[:, c * M:(c + 1) * M])})
        l3_st.append(q3)
        l3_qr.append(qr)      # ranks filled in after L2
    return perm2, ys, ws, sched2, sched3, s2max, s3max, l2_maps, l3_st, l3_qr


def l3_fill_ranks(dens, sched3, s3max, l3_qr, perm2):
    """qr[p, h*s3max+s] = quantized rank of point sched3[b][s]*128+p."""
    N_ = len(dens)
    orig = perm2                                   # sorted idx -> original idx
    order = np.lexsort((-orig, dens))              # dens asc, orig desc
    rank = np.empty(N_, np.int64); rank[order] = np.arange(N_)
    qr = np.minimum(rank // 4, 2047).astype(np.float32) + 1.0
    for c in range(NC):
        for h in range(2):
            b = c * 2 + h
            for s, ch in enumerate(sched3[b]):
                l3_qr[c][:, h * s3max + s] = qr[ch * 128:(ch + 1) * 128]
    return qr


def host_finish(idx8_all, ys, dens, sched3, s3max, perm2):
    """Exact argmax among device top-8 candidates (+self) per query."""
    N_ = len(ys)
    orig = perm2
    mode = np.empty(N_, np.int64)
    for c in range(NC):
        idx8 = idx8_all[c]                         # [128, 64]
        for h in range(2):
            b = c * 2 + h
            chunks = sched3[b]
            for sb in range(4):
                cols = idx8[:, (h * 4 + sb) * 8:(h * 4 + sb) * 8 + 8]  # [128, 8]
                for p in range(128):
                    i = c * M + h * 512 + sb * 128 + p
                    jrows = cols[p]
                    cand = (chunks[:, None] * 128 + jrows[None, :]).ravel()
                    cand = np.concatenate([cand, [i]])
                    cand = cand[(cand >= 0) & (cand < N_)]
                    d2c = ((ys[i] - ys[cand]) ** 2).sum(1)
                    ok = cand[d2c <= H2]
                    if len(ok) == 0:
                        ok = np.array([i])
                    best = ok[np.lexsort((orig[ok], -dens[ok]))[0]]
                    mode[i] = best
    return mode


def kernel(q, attn):
    if "l1" not in _cache:
        _cache["l1"] = build_kernel(skip_final=True)
    cores = list(range(NC))

    in_maps = host_prep(q, attn)
    res1 = bass_utils.run_bass_kernel_spmd(_cache["l1"], in_maps, core_ids=cores)
    y_full = np.concatenate([r["y_out"].T for r in res1.results], axis=0)  # [N,3] centered

    (perm2, ys, ws, sched2, sched3, s2max, s3max,
     l2_maps, l3_st, l3_qr) = final_phase_prep(y_full, attn)

    if ("l2", s2max) not in _cache:
        _cache[("l2", s2max)] = build_kernel_l2(s2max)
    res2 = bass_utils.run_bass_kernel_spmd(_cache[("l2", s2max)], l2_maps,
                                           core_ids=cores)
    dens = np.concatenate([r["dens_out"][0] for r in res2.results])

    qr = l3_fill_ranks(dens, sched3, s3max, l3_qr, perm2)
    if ("l3", s3max) not in _cache:
        _cache[("l3", s3max)] = build_kernel_l3(s3max)
    ident = np.eye(128, dtype=np.float16)
    l3_maps = [{"st3": l3_st[c], "ya3": l2_maps[c]["yaf"],
                "qr3": l3_qr[c], "ident": ident} for c in cores]
    res3 = bass_utils.run_bass_kernel_spmd(_cache[("l3", s3max)], l3_maps,
                                           core_ids=cores)
    idx8_all = [r["idx8"] for r in res3.results]

    mode = host_finish(idx8_all, ys, dens, sched3, s3max, perm2)
    out_sorted = ys[mode] + 0.5
    out = np.empty_like(out_sorted)
    out[perm2] = out_sorted
    return out.astype(np.float32)


if __name__ == "__main__":
    import reference as refmod
    inputs = {k: np.asarray(v) for k, v in refmod.setup_inputs().items()}
    expected = np.asarray(refmod.reference(**inputs))
    out = kernel(**inputs)
    rel = np.linalg.norm(out - expected) / np.linalg.norm(expected)
    print("Relative error:", rel)
